# revision 1
# baseline (speedup 1.0000x reference)
"""Multi-head attention Trainium2 kernel (B=8, N=1024, D=512, H=16, DH=64).

Sharding: pure data-parallel over batch — each of the 8 NeuronCores computes
one batch element end-to-end (no collectives needed).

Per-core dataflow ("transposed world", all matmuls bf16, fp32 PSUM accum):
  - host supplies input^T [D, N] and notmask^T [N, N] (bf16)
  - Q^T, K^T [H*DH, N] via matmul(lhsT=W chunk, rhs=input^T); V [N, H*DH]
    stored interleaved as [ones64 | V_h] per head for the fused row-sum
  - per head pair (2 heads of 64 share one 128-partition tile):
      S^T[j,i] tiles via row-tiled K=64 matmul pairs (both heads concurrent
      in the PE array, base partitions 0 / 64)
      P = exp(S^T/8) via one ScalarE activation per [128, 2048] PSUM span
      P *= notmask^T (VectorE bf16 tensor_tensor, 2x mode)
      ctx^T accum: matmul(lhsT=[ones|V_h], rhs=P) -> rows 0-63 = sum_j P
      (softmax denominator, replicated), rows 64-127 = unnormalized ctx^T
      normalize: reciprocal_approx_fast + tensor_mul
  - out^T [DH, N] = sum_h Wo_h^T-chunk contraction over ctx^T; host transposes
"""

import numpy as np
import ml_dtypes

import concourse.bass as bass
import concourse.mybir as mybir
import concourse.tile as tile
from concourse import bacc
from concourse.bass_utils import run_bass_kernel_spmd

BF16 = ml_dtypes.bfloat16
B, N, D, H, DH = 8, 1024, 512, 16, 64
NT = N // 128  # 8 j-chunks
CT = D // 128  # 4 contraction chunks
PAIRS = H // 2  # 8 head pairs
FP32 = mybir.dt.float32
BF = mybir.dt.bfloat16
EXP = mybir.ActivationFunctionType.Exp

_CACHE = {}


def build_attention_nc():
    """Build the single-core bass program (SPMD: same program, 8 cores)."""
    nc = bacc.Bacc()
    inT_d = nc.dram_tensor("inT", [D, N], BF, kind="ExternalInput")
    nmT_d = nc.dram_tensor("nmT", [N, N], BF, kind="ExternalInput")
    wq_d = nc.dram_tensor("wq", [D, H * DH], BF, kind="ExternalInput")
    wk_d = nc.dram_tensor("wk", [D, H * DH], BF, kind="ExternalInput")
    wv_d = nc.dram_tensor("wv", [D, H * DH], BF, kind="ExternalInput")
    wo_d = nc.dram_tensor("wo", [H * DH, DH], BF, kind="ExternalInput")
    outT_d = nc.dram_tensor("outT", [DH, N], FP32, kind="ExternalOutput")

    with tile.TileContext(nc) as tc:
        with (
            tc.tile_pool(name="consts", bufs=1) as consts,
            tc.tile_pool(name="qk", bufs=1) as qkp,
            tc.tile_pool(name="pp", bufs=1) as pp,
            tc.tile_pool(name="cn", bufs=1) as cnp,
            tc.tile_pool(name="rzp", bufs=1) as rzp,
            tc.tile_pool(name="psS", bufs=1, space="PSUM") as psS,
            tc.tile_pool(name="psC", bufs=1, space="PSUM") as psC,
            tc.tile_pool(name="psP", bufs=1, space="PSUM") as psP,
        ):
            # ---- loads ----
            inT = consts.tile([128, CT, N], BF)
            nc.sync.dma_start(inT[:], inT_d[:].rearrange("(c p) n -> p c n", p=128))
            wv = consts.tile([128, CT, H * DH], BF)
            nc.sync.dma_start(wv[:], wv_d[:].rearrange("(c p) m -> p c m", p=128))
            wq = consts.tile([128, CT, H * DH], BF)
            nc.sync.dma_start(wq[:], wq_d[:].rearrange("(c p) m -> p c m", p=128))
            wk = consts.tile([128, CT, H * DH], BF)
            nc.sync.dma_start(wk[:], wk_d[:].rearrange("(c p) m -> p c m", p=128))
            nmT = consts.tile([128, NT, N], BF)
            nc.sync.dma_start(nmT[:], nmT_d[:].rearrange("(t p) n -> p t n", p=128))
            wo = consts.tile([64, H, DH], BF)
            nc.sync.dma_start(wo[:], wo_d[:].rearrange("(h p) e -> p h e", p=64))

            # ---- V projection into [ones64 | V_h] interleaved layout ----
            # vaug[:, jt, h*128:h*128+64] = 1.0 ; [... +64:+128] = V rows
            vaug = consts.tile([128, NT, H * 128], BF)
            nc.gpsimd.memset(vaug[:].rearrange("p t (h x) -> p t h x", x=128)[:, :, :, 0:64], 1.0)
            for jt in range(NT):
                for half in range(2):
                    vps = psP.tile([128, 512], FP32, tag="projps", bufs=2)
                    for c in range(CT):
                        nc.tensor.matmul(
                            vps[:],
                            inT[:, c, jt * 128 : (jt + 1) * 128],
                            wv[:, c, half * 512 : (half + 1) * 512],
                            start=(c == 0),
                            stop=(c == CT - 1),
                        )
                    dst = vaug[:, jt, :].rearrange("p (h x) -> p h x", x=128)[
                        :, half * 8 : (half + 1) * 8, 64:128
                    ]
                    nc.vector.tensor_copy(dst, vps[:].rearrange("p (h x) -> p h x", x=64))

            # ---- QK projections (per pair tile t: 2 heads = 128 out cols) ----
            qts = [None] * PAIRS
            kts = [None] * PAIRS
            ctxn = [None] * H

            def project_pair(t):
                qt = qkp.tile([128, N], BF, tag="qt", bufs=4, name=f"qt{t}")
                kt = qkp.tile([128, N], BF, tag="kt", bufs=4, name=f"kt{t}")
                for dst_t, w in ((qt, wq), (kt, wk)):
                    for half in range(2):
                        pps = psP.tile([128, 512], FP32, tag="projps", bufs=2)
                        for c in range(CT):
                            nc.tensor.matmul(
                                pps[:],
                                w[:, c, t * 128 : (t + 1) * 128],
                                inT[:, c, half * 512 : (half + 1) * 512],
                                start=(c == 0),
                                stop=(c == CT - 1),
                            )
                        nc.vector.tensor_copy(dst_t[:, half * 512 : (half + 1) * 512], pps[:])
                qts[t], kts[t] = qt, kt

            project_pair(0)
            project_pair(1)

            # ---- attention per head pair ----
            for h2 in range(PAIRS):
                qt, kt = qts[h2], kts[h2]
                p_tiles = []
                for jt in range(NT):
                    s_ps = psS.tile([128, 2048], FP32, tag="s", bufs=1, name=f"s{h2}_{jt}")
                    for hh, base in ((0, 0), (1, 1024)):
                        lo, hi = hh * 64, hh * 64 + 64
                        for half in range(2):
                            nc.tensor.matmul(
                                s_ps[:, base + half * 512 : base + (half + 1) * 512],
                                kt[lo:hi, jt * 128 : (jt + 1) * 128],
                                qt[lo:hi, half * 512 : (half + 1) * 512],
                                start=True,
                                stop=True,
                            )
                    p_t = pp.tile([128, 2048], BF, tag="p", bufs=10, name=f"p{h2}_{jt}")
                    nc.scalar.activation(p_t[:], s_ps[:], EXP, scale=0.125)
                    nc.vector.tensor_mul(p_t[:, 0:1024], p_t[:, 0:1024], nmT[:, jt, :])
                    nc.vector.tensor_mul(p_t[:, 1024:2048], p_t[:, 1024:2048], nmT[:, jt, :])
                    p_tiles.append(p_t)

                for hh in range(2):
                    h = 2 * h2 + hh
                    cn_t = cnp.tile([64, N], BF, tag=f"cn{h}", name=f"cn{h}")
                    for half in range(2):
                        cps = psC.tile([128, 512], FP32, tag="ctx", bufs=2)
                        off = hh * 1024 + half * 512
                        for jt in range(NT):
                            nc.tensor.matmul(
                                cps[:],
                                vaug[:, jt, h * 128 : (h + 1) * 128],
                                p_tiles[jt][:, off : off + 512],
                                start=(jt == 0),
                                stop=(jt == NT - 1),
                            )
                        rz = rzp.tile([64, 512], FP32, tag="rz", bufs=4)
                        nc.vector.reciprocal_approx_fast(out=rz[:], in_=cps[0:64, :])
                        nc.vector.tensor_mul(
                            cn_t[:, half * 512 : (half + 1) * 512], cps[64:128, :], rz[:]
                        )
                    ctxn[h] = cn_t

                if h2 + 2 < PAIRS:
                    project_pair(h2 + 2)

            # ---- output projection: outT[e, i] = sum_h Wo_h^T chunks ----
            ops = psS.tile([64, N], FP32, tag="s", bufs=1)
            for h in range(H):
                for half in range(2):
                    nc.tensor.matmul(
                        ops[:, half * 512 : (half + 1) * 512],
                        wo[:, h, :],
                        ctxn[h][:, half * 512 : (half + 1) * 512],
                        start=(h == 0),
                        stop=(h == H - 1),
                    )
            out_sb = consts.tile([64, N], FP32)
            nc.vector.tensor_copy(out_sb[:], ops[:])
            nc.sync.dma_start(outT_d[:], out_sb[:])

    nc.finalize()
    return nc


def _prep_inputs(input, attn_mask, Wq, Wk, Wv, Wo):
    """Host-side shard prep: per-core transposed bf16 views."""
    inp = np.asarray(input)
    mask = np.asarray(attn_mask)
    wq = np.ascontiguousarray(np.asarray(Wq), dtype=np.float32).astype(BF16)
    wk = np.ascontiguousarray(np.asarray(Wk), dtype=np.float32).astype(BF16)
    wv = np.ascontiguousarray(np.asarray(Wv), dtype=np.float32).astype(BF16)
    wo = np.ascontiguousarray(np.asarray(Wo), dtype=np.float32).astype(BF16)
    in_maps = []
    for b in range(B):
        inT = np.ascontiguousarray(inp[b].T).astype(BF16)
        nmT = np.ascontiguousarray(~mask[b].T).astype(BF16)
        in_maps.append(
            {"inT": inT, "nmT": nmT, "wq": wq, "wk": wk, "wv": wv, "wo": wo}
        )
    return in_maps


def kernel(**inputs):
    if "nc" not in _CACHE:
        _CACHE["nc"] = build_attention_nc()
    nc = _CACHE["nc"]
    in_maps = _prep_inputs(
        inputs["input"], inputs["attn_mask"], inputs["Wq"], inputs["Wk"],
        inputs["Wv"], inputs["Wo"],
    )
    res = run_bass_kernel_spmd(nc, in_maps, core_ids=list(range(B)))
    out = np.empty((B, N, DH), dtype=np.float32)
    for b in range(B):
        out[b] = res.results[b]["outT"].T
    return out
